# revision 18
# baseline (speedup 1.0000x reference)
"""Trainium2 Bass kernel v5 for nn_Net_43061342110447 (3-layer GCN + Set2Set head).

Math: zero LSTM biases => Set2Set query is 0 => uniform attention => the head
needs only m = mean_n h3, which pulls back through the linear layers 2/3:
    m = (1/N)[((g2^T h1) W2 + (sum g3) b2) W3] + b3
so the device only computes h1 = relu(D A' D x W1 + b1) and p = g2^T h1.

Device (8 cores):
  - node -> (core, window) by global degree sort, LPT-balanced core deal per
    1024-node block row (equalizes per-(quarter,window) counts across cores)
  - per (quarter, window) edge slots padded to the max count over cores C_qw
    (NOT to multiples of 128); chunks of 128 slots may span window boundaries
    and the scatter matmul splits into per-window partition-range segments
    whose boundaries are core-independent (SPMD-safe) -> near-zero padding
  - gather table T1 = dis*x bf16 256B rows, 4 int16-indexable quarter blocks,
    dma_gather over 4 SWDGE queues; pad slots fetch a zero row
  - one-hot matrices precomputed on host (zero rows for pad slots) and
    streamed via static-queue DMA; segment-sum via PSUM-accumulated matmuls
  - per window (interleaved into last quarter): scale by dis, transpose,
    @W1 bf16, +b1, relu, weighted reduce with g2 into one [1,256] per core
"""
import numpy as np
import ml_dtypes

import concourse.bacc as bacc
import concourse.mybir as mybir
from concourse.tile import TileContext
from concourse.bass_utils import run_bass_kernel_spmd

BF16 = ml_dtypes.bfloat16
FP8 = ml_dtypes.float8_e4m3
P = 128
NCORES = 8
FT = 256  # fp8 elements per 256B table row
QDATA = 25088
QROWS = 25120
ZREL = QDATA
NQUART = 4
TROWS = NQUART * QROWS
BATCH = 64
NSWQ = 4

N_NODES = 100000
F_IN, F1 = 64, 256
NSH = N_NODES // NCORES          # 12500
NW = (NSH + P - 1) // P          # 98
WPAD = NW * P


def _assign_nodes(edge_index):
    """node -> (core, pos) : global degree sort, LPT core balance per block row."""
    src = np.concatenate([edge_index[0], np.arange(N_NODES, dtype=np.int64)])
    dst = np.concatenate([edge_index[1], np.arange(N_NODES, dtype=np.int64)])
    q = (src // QDATA).astype(np.int64)
    degq = np.zeros((N_NODES, NQUART), np.int64)
    np.add.at(degq, (dst, q), 1)
    deg = degq.sum(1)
    order = np.argsort(-deg, kind='stable')

    node_core = np.empty(N_NODES, np.int32)
    node_pos = np.empty(N_NODES, np.int64)
    for w in range(NW):
        rows = order[w * P * NCORES:(w + 1) * P * NCORES]
        dq = degq[rows]
        fills = np.zeros((NCORES, NQUART), np.int64)
        counts = np.zeros(NCORES, np.int64)
        for i, nd in enumerate(rows):
            avail = counts < P
            score = (fills + dq[i]).max(axis=1) * 1000 + fills.sum(axis=1)
            score[~avail] = 1 << 60
            c = int(np.argmin(score))
            node_core[nd] = c
            node_pos[nd] = w * P + counts[c]
            fills[c] += dq[i]
            counts[c] += 1
    return node_core, node_pos


def _host_prep(edge_index):
    # self-loops are NOT slotted: they enter as Z's DMA-loaded initialization
    src = np.asarray(edge_index[0], np.int64)
    dst = np.asarray(edge_index[1], np.int64)

    node_core, node_pos = _assign_nodes(edge_index)
    node_w = (node_pos // P).astype(np.int32)
    node_prel = (node_pos % P).astype(np.int32)

    e_core = node_core[dst]
    e_w = node_w[dst]
    e_prel = node_prel[dst]
    e_q = (src // QDATA).astype(np.int32)
    e_rel = (src % QDATA).astype(np.int16)

    key = (e_core.astype(np.int64) * NQUART + e_q) * NW + e_w
    cnt = np.bincount(key, minlength=NCORES * NQUART * NW).reshape(NCORES, NQUART, NW)
    C_qw = ((cnt.max(axis=0) + 63) // 64) * 64   # spans padded to 64 (PE tile rule)

    # per-quarter contiguous slot layout; chunks of 128 may span windows
    seg_meta = []     # per quarter: list over chunks of [(w, s0, s1, first, last)]
    CQ = []           # chunks per quarter
    span_start = np.zeros((NQUART, NW), np.int64)
    for q in range(NQUART):
        r = 0
        for w in range(NW):
            span_start[q, w] = r
            r += int(C_qw[q, w])
        nch = (r + P - 1) // P
        CQ.append(nch)
        segs = [[] for _ in range(nch)]

        def legal_pieces(a, b):
            # 64-aligned spans: pieces start at 0 or 64 only, both legal
            assert a in (0, 64)
            return [(a, b)]

        for w in range(NW):
            s0, s1 = int(span_start[q, w]), int(span_start[q, w] + C_qw[q, w])
            if s1 == s0:
                continue
            pieces = []
            for ci in range(s0 // P, (s1 - 1) // P + 1):
                a = max(s0, ci * P) - ci * P
                b = min(s1, (ci + 1) * P) - ci * P
                for (pa, pb) in legal_pieces(a, b):
                    pieces.append((ci, pa, pb))
            for k, (ci, pa, pb) in enumerate(pieces):
                segs[ci].append((w, pa, pb, k == 0, k == len(pieces) - 1))
        seg_meta.append(segs)
    CQ = np.array(CQ)
    CQoff = np.concatenate([[0], np.cumsum(CQ)]).astype(np.int64)
    nchunks = int(CQoff[-1])
    nslots = nchunks * P

    order_all = np.lexsort((e_w, e_q, e_core))
    bounds = np.searchsorted(e_core[order_all], np.arange(NCORES + 1))
    per_core = []
    for c in range(NCORES):
        sel = order_all[bounds[c]:bounds[c + 1]]
        qidx = np.full(nslots, ZREL, np.int16)
        drel = np.full(nslots, -1, np.int32)
        gkey = e_q[sel] * NW + e_w[sel]
        gb = np.searchsorted(gkey, np.arange(NQUART * NW + 1))
        within = np.arange(len(sel)) - np.repeat(gb[:-1], np.diff(gb))
        slots = (CQoff[e_q[sel]] * P + span_start[e_q[sel], e_w[sel]] + within)
        qidx[slots] = e_rel[sel]
        drel[slots] = e_prel[sel]
        idx16 = np.tile(qidx.reshape(-1, 16).T, (8, 1))
        drel2 = drel.reshape(-1, P).T                     # [128, nchunks]
        oh = (drel2[:, :, None] == np.arange(P)[None, None, :]).astype(FP8)
        per_core.append(dict(idx16=idx16, oh=oh))

    meta = dict(seg_meta=seg_meta, CQ=CQ, CQoff=CQoff,
                nchunks=nchunks, nslots=nslots)
    return meta, per_core, node_core, node_pos


def _build_kernel(meta):
    fp32 = mybir.dt.float32
    bf16 = mybir.dt.bfloat16
    nc = bacc.Bacc("TRN2", target_bir_lowering=False, debug=False,
                   num_devices=NCORES, num_swdge_queues=NSWQ)
    nchunks, nslots = meta["nchunks"], meta["nslots"]

    fp8 = mybir.dt.float8e4
    T1_ext = nc.declare_dram_parameter("T1", [TROWS, FT], fp8, isOutput=False)
    W1_ext = nc.declare_dram_parameter("W1t", [F_IN, F1], bf16, isOutput=False)
    b1_ext = nc.declare_dram_parameter("b1t", [P, F1], fp32, isOutput=False)
    dis_ext = nc.declare_dram_parameter("dis_t", [P, NW], fp32, isOutput=False)
    cvec_ext = nc.declare_dram_parameter("cvec_t", [P, NW], bf16, isOutput=False)
    ident_ext = nc.declare_dram_parameter("ident_t", [P, P], bf16, isOutput=False)
    idx_ext = nc.declare_dram_parameter("idx16", [P, nslots // 16], mybir.dt.int16, isOutput=False)
    d2x_ext = nc.declare_dram_parameter("d2x_t", [P, NW, F_IN], fp32, isOutput=False)
    CQoff_l = [int(x) for x in meta["CQoff"]]
    oh_ext = nc.declare_dram_parameter("oh_t", [P, nchunks, P], fp8, isOutput=False)
    pvec_ext = nc.declare_dram_parameter("pvec", [1, F1], fp32, isOutput=True)

    call_i = 0
    with TileContext(nc) as tc:
        with tc.tile_pool(name="const", bufs=1) as cpool, \
             tc.tile_pool(name="zbuf", bufs=1) as zpool, \
             tc.tile_pool(name="msg", bufs=4) as mpool, \
             tc.tile_pool(name="oh", bufs=4) as ohpool, \
             tc.tile_pool(name="work", bufs=3) as wpool, \
             tc.tile_pool(name="aggps", bufs=5, space="PSUM") as aggps, \
             tc.tile_pool(name="tps", bufs=1, space="PSUM") as tpsp, \
             tc.tile_pool(name="mmps", bufs=1, space="PSUM") as mmpsp, \
             tc.tile_pool(name="rps", bufs=1, space="PSUM") as rpsp:

            W1_t = cpool.tile([F_IN, F1], bf16)
            b1_t = cpool.tile([P, F1], fp32)
            dis_t = cpool.tile([P, NW], fp32)
            cvec_t = cpool.tile([P, NW], bf16)
            ident_t = cpool.tile([P, P], bf16)
            HEADC = 64  # chunks served from the small head tile
            idx_h, idx_r = [], []
            for qq in range(NQUART):
                ncols = (CQoff_l[qq + 1] - CQoff_l[qq]) * 8
                hc = min(ncols, HEADC * 8)
                th = cpool.tile([P, max(hc, 8)], mybir.dt.int16, tag=f"idxh{qq}", name=f"idxh{qq}")
                tr = cpool.tile([P, max(ncols - hc, 8)], mybir.dt.int16, tag=f"idxr{qq}", name=f"idxr{qq}")
                idx_h.append(th)
                idx_r.append(tr)

            for qq in range(NQUART):
                c0, c1 = CQoff_l[qq] * 8, CQoff_l[qq + 1] * 8
                hc = min(c1 - c0, HEADC * 8)
                if hc > 0:
                    eng = nc.sync if qq % 2 == 0 else nc.scalar
                    eng.dma_start(out=idx_h[qq][:, :hc], in_=idx_ext[:, c0:c0 + hc])
            for qq in range(NQUART):
                c0, c1 = CQoff_l[qq] * 8, CQoff_l[qq + 1] * 8
                hc = min(c1 - c0, HEADC * 8)
                if c1 - c0 - hc > 0:
                    eng = nc.sync if qq % 2 == 0 else nc.scalar
                    eng.dma_start(out=idx_r[qq][:, :c1 - c0 - hc], in_=idx_ext[:, c0 + hc:c1])
            nc.scalar.dma_start(out=W1_t[:], in_=W1_ext[:, :])
            nc.scalar.dma_start(out=b1_t[:], in_=b1_ext[:, :])
            nc.scalar.dma_start(out=dis_t[:], in_=dis_ext[:, :])
            nc.scalar.dma_start(out=cvec_t[:], in_=cvec_ext[:, :])
            nc.scalar.dma_start(out=ident_t[:], in_=ident_ext[:, :])

            Z = zpool.tile([P, NW, F_IN], fp32)
            nc.sync.dma_start(out=Z[:], in_=d2x_ext[:, :, :])

            pp = rpsp.tile([1, F1], fp32)

            tcount = [0]

            def transform(w):
                ti = tcount[0]
                tcount[0] += 1
                dcol = dis_t[:, w:w + 1]
                a = wpool.tile([P, F_IN], bf16, tag="ta")
                nc.scalar.activation(a[:], Z[:, w, :],
                                     mybir.ActivationFunctionType.Copy, scale=dcol)
                tp = tpsp.tile([P, P], bf16, tag="tps")
                nc.tensor.transpose(out=tp[:F_IN, :], in_=a[:], identity=ident_t[:])
                aT = wpool.tile([F_IN, P], bf16, tag="taT")
                nc.any.tensor_copy(out=aT[:], in_=tp[:F_IN, :])
                ps2 = mmpsp.tile([P, F1], fp32, tag="mmps")
                nc.tensor.matmul(out=ps2[:], lhsT=aT[:], rhs=W1_t[:],
                                 start=True, stop=True)
                h = wpool.tile([P, F1], bf16, tag="th")
                nc.vector.tensor_tensor(out=h[:], in0=ps2[:], in1=b1_t[:],
                                        op=mybir.AluOpType.add)
                nc.scalar.activation(h[:], h[:], mybir.ActivationFunctionType.Relu)
                nc.tensor.matmul(out=pp[:], lhsT=cvec_t[:, w:w + 1], rhs=h[:],
                                 start=(ti == 0), stop=(ti == NW - 1))

            # striped emission: round-robin calls across the 4 quarter streams
            # so every window's last (q3) chunk lands ~in proportion to w and
            # transforms spread across the whole gather phase
            psum_by_qw = {}
            wq_total = [0] * NW     # quarters with a nonzero span per window
            for q in range(NQUART):
                for segs_c in meta["seg_meta"][q]:
                    for (w, a, b, first, last) in segs_c:
                        if last:
                            wq_total[w] += 1
            wq_done = [0] * NW
            cursors = [0] * NQUART
            active = [q for q in range(NQUART) if int(meta["CQ"][q]) > 0]
            while active:
                for q in list(active):
                    cq = int(meta["CQ"][q])
                    b0 = cursors[q]
                    if b0 >= cq:
                        active.remove(q)
                        continue
                    rem = cq - b0
                    nb = min(BATCH, rem) if rem > 48 else min(16, rem)
                    cursors[q] += nb
                    coff = int(meta["CQoff"][q])
                    segs = meta["seg_meta"][q]
                    tbl_q = T1_ext[q * QROWS:(q + 1) * QROWS, :]
                    msg = mpool.tile([P, BATCH, FT], fp8, tag="msg")
                    if b0 < HEADC:
                        idx_src = idx_h[q]
                        icol0 = b0 * 8
                    else:
                        idx_src = idx_r[q]
                        icol0 = (b0 - HEADC) * 8
                    nc.gpsimd.dma_gather(
                        out_ap=msg[:, :nb, :], in_ap=tbl_q,
                        idxs_ap=idx_src[:, icol0:icol0 + nb * 8],
                        num_idxs=nb * P, num_idxs_reg=nb * P,
                        elem_size=FT, single_packet=False,
                        queue_num=q % NSWQ)
                    call_i += 1
                    ohx = ohpool.tile([P, BATCH, P], fp8, tag="oh")
                    oheng = nc.sync if call_i % 2 == 0 else nc.scalar
                    oheng.dma_start(out=ohx[:, :nb, :],
                                    in_=oh_ext[:, coff + b0:coff + b0 + nb, :])
                    for ci in range(nb):
                        for (w, a, b, first, last) in segs[b0 + ci]:
                            if first:
                                psum_by_qw[(q, w)] = aggps.tile([P, F_IN], fp32, tag="aggps", name="aggtile")
                            nc.tensor.matmul(
                                out=psum_by_qw[(q, w)][:],
                                lhsT=ohx[a:b, ci, :], rhs=msg[a:b, ci, :F_IN],
                                start=first, stop=last)
                            if last:
                                nc.vector.tensor_tensor(
                                    out=Z[:, w, :], in0=Z[:, w, :],
                                    in1=psum_by_qw[(q, w)][:], op=mybir.AluOpType.add)
                                del psum_by_qw[(q, w)]
                                wq_done[w] += 1
                                if wq_done[w] == wq_total[w]:
                                    transform(w)

            psb = wpool.tile([1, F1], fp32, tag="psb")
            nc.any.tensor_copy(out=psb[:], in_=pp[:])
            nc.sync.dma_start(out=pvec_ext[:, :], in_=psb[:])

    return nc


_KERNEL_CACHE = {}


def _get_kernel(meta):
    key = (meta["nchunks"], tuple(int(x) for x in meta["CQ"]))
    if key not in _KERNEL_CACHE:
        nc = _build_kernel(meta)
        nc.compile()
        _KERNEL_CACHE[key] = nc
    return _KERNEL_CACHE[key]


def _head(m, inputs):
    def sigmoid(v):
        return 1.0 / (1.0 + np.exp(-v))

    q_star = np.zeros((1, 64))
    hs = np.zeros((1, 32))
    cs = np.zeros((1, 32))
    gates = (q_star @ np.asarray(inputs["Wih"], np.float64).T
             + hs @ np.asarray(inputs["Whh"], np.float64).T
             + np.asarray(inputs["bih"], np.float64)
             + np.asarray(inputs["bhh"], np.float64))
    i_g, f_g, g_g, o_g = np.split(gates, 4, axis=-1)
    cs = sigmoid(f_g) * cs + sigmoid(i_g) * np.tanh(g_g)
    q = sigmoid(o_g) * np.tanh(cs)
    r = m[None, :]
    q_star = np.concatenate([q, r], axis=-1)
    out = np.maximum(q_star @ np.asarray(inputs["Wl1"], np.float64)
                     + np.asarray(inputs["bl1"], np.float64), 0.0)
    out = out @ np.asarray(inputs["Wl2"], np.float64) + np.asarray(inputs["bl2"], np.float64)
    out = out @ np.asarray(inputs["Wl3"], np.float64) + np.asarray(inputs["bl3"], np.float64)
    out = out @ np.asarray(inputs["Wl4"], np.float64) + np.asarray(inputs["bl4"], np.float64)
    return out.reshape(-1).astype(np.float32), q


def _fallback_numpy(inputs):
    x = np.asarray(inputs["x"], np.float64)
    ei = np.asarray(inputs["edge_index"], np.int64)
    N = x.shape[0]
    row = np.concatenate([ei[0], np.arange(N)])
    col = np.concatenate([ei[1], np.arange(N)])
    deg = np.bincount(col, minlength=N).astype(np.float64)
    dis = 1.0 / np.sqrt(deg)
    norm = dis[row] * dis[col]

    def gcn(h, W, b):
        h = h @ np.asarray(W, np.float64)
        out = np.zeros((N, h.shape[1]))
        np.add.at(out, col, h[row] * norm[:, None])
        return out + np.asarray(b, np.float64)

    h = np.maximum(gcn(x, inputs["W1"], inputs["b1"]), 0.0)
    h = gcn(h, inputs["W2"], inputs["b2"])
    h = gcn(h, inputs["W3"], inputs["b3"])

    def sigmoid(v):
        return 1.0 / (1.0 + np.exp(-v))

    q_star = np.zeros((1, 64))
    hs = np.zeros((1, 32))
    cs = np.zeros((1, 32))
    gates = (q_star @ np.asarray(inputs["Wih"], np.float64).T
             + hs @ np.asarray(inputs["Whh"], np.float64).T
             + np.asarray(inputs["bih"], np.float64)
             + np.asarray(inputs["bhh"], np.float64))
    i_g, f_g, g_g, o_g = np.split(gates, 4, axis=-1)
    cs = sigmoid(f_g) * cs + sigmoid(i_g) * np.tanh(g_g)
    q = sigmoid(o_g) * np.tanh(cs)
    e = h @ q[0]
    a = np.exp(e - e.max())
    a /= a.sum()
    r = (a[:, None] * h).sum(axis=0)[None, :]
    q_star = np.concatenate([q, r], axis=-1)
    out = np.maximum(q_star @ np.asarray(inputs["Wl1"], np.float64)
                     + np.asarray(inputs["bl1"], np.float64), 0.0)
    out = out @ np.asarray(inputs["Wl2"], np.float64) + np.asarray(inputs["bl2"], np.float64)
    out = out @ np.asarray(inputs["Wl3"], np.float64) + np.asarray(inputs["bl3"], np.float64)
    out = out @ np.asarray(inputs["Wl4"], np.float64) + np.asarray(inputs["bl4"], np.float64)
    return out.reshape(-1).astype(np.float32)


def kernel(x, edge_index, W1, b1, W2, b2, W3, b3,
           Wih, Whh, bih, bhh, Wl1, bl1, Wl2, bl2, Wl3, bl3, Wl4, bl4):
    inputs = dict(x=x, edge_index=edge_index, W1=W1, b1=b1, W2=W2, b2=b2,
                  W3=W3, b3=b3, Wih=Wih, Whh=Whh, bih=bih, bhh=bhh,
                  Wl1=Wl1, bl1=bl1, Wl2=Wl2, bl2=bl2, Wl3=Wl3, bl3=bl3,
                  Wl4=Wl4, bl4=bl4)
    _, q = _head(np.zeros(32), inputs)
    if np.abs(q).max() != 0.0 or x.shape[0] != N_NODES:
        return _fallback_numpy(inputs)

    x = np.asarray(x, np.float32)
    edge_index = np.asarray(edge_index, np.int64)
    N = N_NODES

    col = np.concatenate([edge_index[1], np.arange(N, dtype=np.int64)])
    row = np.concatenate([edge_index[0], np.arange(N, dtype=np.int64)])
    deg = np.bincount(col, minlength=N).astype(np.float64)
    dis = 1.0 / np.sqrt(deg)
    g3 = dis * np.bincount(row, weights=dis[col], minlength=N)
    g2 = dis * np.bincount(row, weights=(dis * g3)[col], minlength=N)

    meta, per_core, node_core, node_pos = _host_prep(edge_index)
    nc = _get_kernel(meta)

    T1 = np.zeros((TROWS, FT), FP8)
    xs_f32 = x * dis[:, None].astype(np.float32)
    xs = xs_f32.astype(FP8)
    for qq in range(NQUART):
        lo, hi = qq * QDATA, min((qq + 1) * QDATA, N)
        if hi > lo:
            T1[qq * QROWS: qq * QROWS + (hi - lo), :F_IN] = xs[lo:hi]

    ident = np.eye(P, dtype=np.float32)
    common = dict(T1=T1, W1t=np.asarray(W1, np.float32).astype(BF16),
                  b1t=np.tile(np.asarray(b1, np.float32), (P, 1)),
                  ident_t=ident.astype(BF16))
    in_maps = []
    for c in range(NCORES):
        sel = node_core == c
        pos = node_pos[sel]
        dis_sh = np.zeros((WPAD,), np.float32)
        dis_sh[pos] = dis[sel]
        c_sh = np.zeros((WPAD,), np.float32)
        c_sh[pos] = g2[sel]
        d2x = np.zeros((WPAD, F_IN), np.float32)
        d2x[pos] = xs_f32[sel]
        in_maps.append(dict(common,
                            dis_t=dis_sh.reshape(NW, P).T.copy(),
                            cvec_t=c_sh.reshape(NW, P).T.astype(BF16).copy(),
                            idx16=per_core[c]["idx16"],
                            oh_t=per_core[c]["oh"],
                            d2x_t=d2x.reshape(NW, P, F_IN).transpose(1, 0, 2).copy()))

    res = run_bass_kernel_spmd(nc, in_maps, core_ids=list(range(NCORES)))

    p = np.zeros(F1, np.float64)
    for c in range(NCORES):
        p += res.results[c]["pvec"].reshape(-1).astype(np.float64)

    g3h2 = p @ np.asarray(W2, np.float64) + g3.sum() * np.asarray(b2, np.float64)
    m = (g3h2 @ np.asarray(W3, np.float64)) / N + np.asarray(b3, np.float64)
    out, _ = _head(m, inputs)
    return out

